# revision 16
# baseline (speedup 1.0000x reference)
"""ASCADv2 head kernel for Trainium2 (8 NeuronCores, pure data parallel).

Algorithm (per batch element b; reference computes):
  probs = softmax(logits, -1); alpha=probs[0], beta=probs[1], ms=probs[2:]
  xorred[l,z] = sum_x ms[l,x] * beta[x^z]            (XOR convolution)
  out[l,z]    = sum_{x*y=z in GF(256)} inv_alpha[x] * xorred[l,y]
  return log(clip(out, 1e-12))

Key transforms used here:
  * XOR convolution diagonalizes under the Walsh-Hadamard transform H
    (constant +-1 256x256 matrix): xorred = H(( H m ) .* ( H beta ))/256.
  * The GF(256)* multiplicative convolution is a length-255 cyclic
    convolution in the discrete-log domain (generator g=3), diagonalized
    by a DFT-255 implemented as constant cos/sin matmuls; real-input
    conjugate symmetry halves the spectrum to k=0..127.
  * Softmax normalizers are factored out of the bilinear pipeline and
    re-applied as a per-row scale inside the final log (ACT: Ln(U*scale)).
  * z=0 column (the GF multiply-by-zero mass) is patched separately:
    out[l,0] = inv_a[0]*(sum_{y!=0} xorred[l,y]) + (sum inv_a)*xorred[l,0]
    with xorred[l,0] = <ms[l], beta> computed as a fused DVE dot product.

Layouts: softmax+log in rows-on-partition layout; all matmuls contract
over z so the bulk pipeline runs z-on-partition; PE transposes convert.
"""

import numpy as np

import concourse.bass as bass
import concourse.bacc as bacc
import concourse.tile as tile
import concourse.mybir as mybir
from concourse.bass_utils import run_bass_kernel_spmd

F32 = mybir.dt.float32
F32R = mybir.dt.float32r
AF = mybir.ActivationFunctionType
ALU = mybir.AluOpType

N_CORES = 8
B_TOTAL = 2048
BC = B_TOTAL // N_CORES  # batches per core

LOG_CLIP = float(np.log(np.float32(1e-12)))


# ----------------------------------------------------------------------------
# host-side constant tables
# ----------------------------------------------------------------------------

def _gf_tables():
    AES_POLY = 0x1B
    a = np.arange(256, dtype=np.int64)
    x = np.repeat(a, 256)
    y = np.tile(a, 256)
    r = np.zeros(256 * 256, dtype=np.int64)
    for _ in range(8):
        r ^= np.where((y & 1) != 0, x, 0)
        hi = (x & 0x80) != 0
        x = ((x << 1) & 0xFF) ^ np.where(hi, AES_POLY, 0)
        y >>= 1
    mult = r.reshape(256, 256)
    inv = np.argmax(mult == 1, axis=1)
    inv[0] = 0
    return mult, inv


def _consts():
    mult, inv = _gf_tables()
    # powers of generator 3 of GF(256)*
    powers = np.zeros(255, dtype=np.int64)
    v = 1
    for m in range(255):
        powers[m] = v
        v = mult[v, 3]
    assert v == 1

    H = np.array([[1.0]], dtype=np.float32)
    for _ in range(8):
        H = np.block([[H, H], [H, -H]]).astype(np.float32)

    # inverse WHT with dlog ordering fused: col m<255 -> xorred[g^m], col 255 -> xorred[0]
    HINVP = np.empty((256, 256), dtype=np.float32)
    HINVP[:, :255] = H[:, powers] / 256.0
    HINVP[:, 255] = H[:, 0] / 256.0

    # alpha permutation: Ag[k] = alpha[inv(g^k)]
    PINVP = np.zeros((256, 256), dtype=np.float32)
    for k in range(255):
        PINVP[inv[powers[k]], k] = 1.0
    PINVP[inv[0], 255] = 1.0  # unused row-255 output

    # forward DFT-255, half spectrum: cols 0..127 = cos, 128..255 = sin
    kf = np.arange(128)[None, :]
    j = np.arange(255)[:, None]
    CS = np.empty((255, 256), dtype=np.float32)
    CS[:, :128] = np.cos(2 * np.pi * j * kf / 255)
    CS[:, 128:] = np.sin(2 * np.pi * j * kf / 255)

    # inverse DFT with z-ordering fused; factor 2 for folded conjugate half
    m2 = np.arange(255)[None, :]
    kk = np.arange(128)[:, None]
    w = np.full((128, 1), 2.0, dtype=np.float32)
    w[0] = 1.0
    Ci = (w * np.cos(2 * np.pi * kk * m2 / 255) / 255).astype(np.float32)
    Si = (w * np.sin(2 * np.pi * kk * m2 / 255) / 255).astype(np.float32)
    CINVZ = np.zeros((128, 256), dtype=np.float32)
    SINVZ = np.zeros((128, 256), dtype=np.float32)
    CINVZ[:, powers] = Ci
    SINVZ[:, powers] = Si

    IDT = np.eye(128, dtype=np.float32)
    return dict(H=H, HINVP=HINVP, PINVP=PINVP, CS=CS, CINVZ=CINVZ,
                SINVZ=SINVZ, IDT=IDT)


# ----------------------------------------------------------------------------
# kernel emission
# ----------------------------------------------------------------------------

def _emit(tc, out_ap, logits_ap, cdram, bc, stage=None):
    nc = tc.nc
    nh = bc // 128            # 128-row chunks per l
    nt = 18 * nh              # (l,h) tiles
    nn = (16 * bc) // 512     # 512-wide column chunks over msbox
    tpc = 512 // 256          # hmm unused
    lper = 512 // bc
    v3 = lambda ap: ap.rearrange("p (a b) -> p a b", a=lper)
    bcast = lambda ap: ap.unsqueeze(1).broadcast_to([128, lper, bc])

    _cms = []
    def _reg(cm):
        _cms.append(cm)
        return cm
    def _close_all():
        for cm in reversed(_cms):
            try:
                cm.__exit__(None, None, None)
            except Exception:
                pass

    cpool_cm = _reg(tc.tile_pool(name="consts", bufs=1)); cpool = cpool_cm.__enter__()
    sp_cm = _reg(tc.tile_pool(name="small", bufs=1)); sp = sp_cm.__enter__()
    pcsp_cm = _reg(tc.tile_pool(name="pcs", bufs=1)); pcsp = pcsp_cm.__enter__()
    xgp_cm = _reg(tc.tile_pool(name="xg", bufs=1)); xgp = xgp_cm.__enter__()
    vvp_cm = _reg(tc.tile_pool(name="vv", bufs=1)); vvp = vvp_cm.__enter__()
    etp_cm = _reg(tc.tile_pool(name="et", bufs=1)); etp = etp_cm.__enter__()

    def cload(name, rows, cols, src, dt=F32R):
        t = cpool.tile([rows, cols], dt, tag=name, name=name)
        nc.sync.dma_start(out=t, in_=src.bitcast(dt) if dt == F32R else src)
        return t

    HGd, HIPd, PIPd, CSd = cdram["H"].ap(), cdram["HINVP"].ap(), cdram["PINVP"].ap(), cdram["CS"].ap()
    HG = [cload(f"hg{k}", 128, 256, HGd[k * 128:(k + 1) * 128, :]) for k in range(2)]
    HIP = [cload(f"hip{k}", 128, 256, HIPd[k * 128:(k + 1) * 128, :]) for k in range(2)]
    PIP = [cload(f"pip{k}", 128, 256, PIPd[k * 128:(k + 1) * 128, :]) for k in range(2)]
    CSk = [cload("cs0", 128, 256, CSd[0:128, :]), cload("cs1", 127, 256, CSd[128:255, :])]
    CIZ = cload("ciz", 128, 256, cdram["CINVZ"].ap())
    SIZ = cload("siz", 128, 256, cdram["SINVZ"].ap())
    IDT = cload("idt", 128, 128, cdram["IDT"].ap(), dt=F32)

    # per-(l,h)-tile scalars, filled by tiny DMAs later
    Za_t = sp.tile([128, nh], F32, tag="Za_t")
    ZmZb = sp.tile([128, 16 * nh], F32, tag="ZmZb")
    X0R = sp.tile([128, 16 * nh], F32, tag="X0R")
    A0s = sp.tile([128, nh], F32, tag="A0s")

    # exp'd logits, z-on-partition: col = t*256 + zc*128 + p  (t = l*nh + h)
    ETmix = etp.tile([128, nt, 2, 128], F32R, tag="ETmix")
    ETk = [ETmix[:, :, k, :] for k in range(2)]   # [128, nt, 128] views

    def et_cols(k, t0, t1):
        return ETk[k][:, t0:t1, :]

    # ---------------- load, transpose raw logits, exp into z-layout ---------
    with tc.tile_pool(name="xin", bufs=2) as xp, \
         tc.tile_pool(name="pstr", bufs=6, space="PSUM") as pstr:
        xrows = []
        for h in range(nh):
            X = xp.tile([128, 18, 256], F32, tag="X")
            nc.sync.dma_start(out=X, in_=logits_ap[h * 128:(h + 1) * 128, :, :])
            xrows.append(X)
        pairs = [(l * nh + h, l, h) for l in range(18) for h in range(nh)]
        for i in range(0, len(pairs), 2):
            ps = pstr.tile([128, 512], F32, tag="tr")
            for j, (t, l, h) in enumerate(pairs[i:i + 2]):
                for zc in range(2):
                    nc.tensor.transpose(
                        ps[:, j * 256 + zc * 128: j * 256 + (zc + 1) * 128],
                        xrows[h][:, l, zc * 128:(zc + 1) * 128], IDT)
            t0p = pairs[i][0]
            nc.scalar.activation(out=ETmix[:, t0p:t0p + 2, :, :],
                                 in_=ps[:, 0:512], func=AF.Exp)

    # ---- beta WHT, alpha perm + DFT, msbox WHT * Wb -> V --------------------
    Wb = [sp.tile([128, bc], F32, tag=f"Wb{m}", name=f"Wb{m}") for m in range(2)]
    Ag = [sp.tile([128, bc], F32R, tag=f"Ag{m}", name=f"Ag{m}") for m in range(2)]
    Gc = sp.tile([128, bc], F32, tag="Gc")
    Gs = sp.tile([128, bc], F32, tag="Gs")
    V = [vvp.tile([128, 16 * bc], F32R, tag=f"V{m}", name=f"V{m}") for m in range(2)]

    with tc.tile_pool(name="psmm", bufs=3, space="PSUM") as psmm:
        # alpha column-sum (Za) via ones column of H (M=1 matmul), DMA to Zrows
        psA = psmm.tile([1, bc], F32, tag="dc", name="psA", bufs=1)
        for k in range(2):
            nc.tensor.matmul(psA, HG[k][:, 0:1], et_cols(k, 0, nh),
                             start=(k == 0), stop=(k == 1))
        zarow = sp.tile([1, bc], F32, tag="zarow")
        nc.scalar.copy(out=zarow, in_=psA)
        for t in range(nh):
            nc.sync.dma_start(out=Za_t[:, t:t + 1],
                              in_=zarow[0:1, t * 128:(t + 1) * 128])

        for m in range(2):
            msl = slice(m * 128, (m + 1) * 128)
            ps = psmm.tile([128, bc], F32, tag="mmb", bufs=2)
            for k in range(2):
                nc.tensor.matmul(ps, HG[k][:, msl], et_cols(k, nh, 2 * nh),
                                 start=(k == 0), stop=(k == 1))
            nc.scalar.copy(out=Wb[m], in_=ps)

        for m in range(2):
            msl = slice(m * 128, (m + 1) * 128)
            ps = psmm.tile([128, bc], F32, tag="mmb", bufs=2)
            for k in range(2):
                nc.tensor.matmul(ps, PIP[k][:, msl], et_cols(k, 0, nh),
                                 start=(k == 0), stop=(k == 1))
            nc.scalar.copy(out=Ag[m], in_=ps)

        for dst, csl in ((Gc, slice(0, 128)), (Gs, slice(128, 256))):
            ps = psmm.tile([128, bc], F32, tag="mmb", bufs=2)
            nc.tensor.matmul(ps, CSk[0][:, csl], Ag[0],
                             start=True, stop=False)
            nc.tensor.matmul(ps, CSk[1][:, csl], Ag[1][0:127, :],
                             start=False, stop=True)
            nc.scalar.copy(out=dst, in_=ps)

        tpn = 512 // 128     # (l,h)-tiles per 512-col chunk
        for n in range(nn):
            nsl = slice(n * 512, (n + 1) * 512)
            t0 = 2 * nh + n * tpn
            for m in range(2):
                msl = slice(m * 128, (m + 1) * 128)
                ps = psmm.tile([128, 512], F32, tag="mm", bufs=5)
                for k in range(2):
                    nc.tensor.matmul(ps, HG[k][:, msl], et_cols(k, t0, t0 + tpn),
                                     start=(k == 0), stop=(k == 1))
                nc.vector.tensor_mul(v3(V[m][:, nsl]), v3(ps), bcast(Wb[m]))
                if m == 0:
                    # V[0] row 0 (zhat=0 DC) = Zm * Zb, needed for corrections
                    for q in range(4):
                        t = n * 4 + q
                        nc.sync.dma_start(
                            out=ZmZb[:, t:t + 1],
                            in_=V[0][0:1, t * 128:(t + 1) * 128].bitcast(F32))

    etp_cm.__exit__(None, None, None)  # ETmix dead
    _cms.remove(etp_cm)

    # ---- inverse WHT (dlog-ordered) -> Xg ----------------------------------
    Xg = [xgp.tile([128, 16 * bc], F32R, tag=f"Xg{m}", name=f"Xg{m}") for m in range(2)]
    with tc.tile_pool(name="psmm2", bufs=6, space="PSUM") as psmm:
        for n in range(nn):
            nsl = slice(n * 512, (n + 1) * 512)
            for m in range(2):
                msl = slice(m * 128, (m + 1) * 128)
                ps = psmm.tile([128, 512], F32, tag="mm", bufs=5)
                for k in range(2):
                    nc.tensor.matmul(ps, HIP[k][:, msl], V[k][:, nsl],
                                     start=(k == 0), stop=(k == 1))
                if m == 0:
                    nc.scalar.copy(out=Xg[m][:, nsl], in_=ps)
                else:
                    nc.vector.tensor_copy(out=Xg[m][:, nsl], in_=ps)
                    # X0 (= xorred at y=0) lives in Xg row 255
                    for q in range(4):
                        t = n * 4 + q
                        nc.sync.dma_start(
                            out=X0R[:, t:t + 1],
                            in_=Xg[1][127:128, t * 128:(t + 1) * 128].bitcast(F32))

    vvp_cm.__exit__(None, None, None)  # V dead
    _cms.remove(vvp_cm)

    # ---- DFT + pointwise complex multiply -> Pc, Ps ------------------------
    Pc = pcsp.tile([128, 16 * bc], F32R, tag="Pc")
    Ps = pcsp.tile([128, 16 * bc], F32R, tag="Ps")
    with tc.tile_pool(name="psmm3", bufs=4, space="PSUM") as psmm, \
         tc.tile_pool(name="tmp4", bufs=4) as t4p:
        for n in range(nn):
            nsl = slice(n * 512, (n + 1) * 512)
            psC = psmm.tile([128, 512], F32, tag="mmC")
            nc.tensor.matmul(psC, CSk[0][:, 0:128], Xg[0][:, nsl],
                             start=True, stop=False)
            nc.tensor.matmul(psC, CSk[1][:, 0:128], Xg[1][0:127, nsl],
                             start=False, stop=True)
            psS = psmm.tile([128, 512], F32, tag="mmS")
            nc.tensor.matmul(psS, CSk[0][:, 128:256], Xg[0][:, nsl],
                             start=True, stop=False)
            nc.tensor.matmul(psS, CSk[1][:, 128:256], Xg[1][0:127, nsl],
                             start=False, stop=True)
            t1 = t4p.tile([128, 512], F32, tag="t1")
            t2 = t4p.tile([128, 512], F32, tag="t2")
            t3 = t4p.tile([128, 512], F32, tag="t3")
            t4 = t4p.tile([128, 512], F32, tag="t4")
            nc.vector.tensor_mul(v3(t1), v3(psC), bcast(Gc))
            nc.vector.tensor_mul(v3(t2), v3(psS), bcast(Gs))
            nc.vector.tensor_mul(v3(t3), v3(psC), bcast(Gs))
            nc.vector.tensor_mul(v3(t4), v3(psS), bcast(Gc))
            nc.gpsimd.tensor_sub(Pc[:, nsl], t1, t2)
            nc.gpsimd.tensor_add(Ps[:, nsl], t3, t4)

    xgp_cm.__exit__(None, None, None)  # Xg dead
    _cms.remove(xgp_cm)

    # A0s: alpha z=0 row from... gone with ETmix; instead use Ag row for k
    # with g^k == inv? Simplest correct source: logits alpha col 0 exp'd was
    # ETmix[0:1, 0:nh, 0, :]; since ETmix is freed above, use inv_alpha row:
    # inv_alpha[0] = alpha[inv(0)=0] = Ag row with PINVP[...,255]... row 255
    # of Ag = inv_alpha[0] exactly. Ag[1] partition 127.
    for t in range(nh):
        nc.sync.dma_start(out=A0s[:, t:t + 1],
                          in_=Ag[1][127:128, t * 128:(t + 1) * 128].bitcast(F32))

    # ---- corrections (batched, rows layout) --------------------------------
    b16 = lambda ap: ap.rearrange("p (a b) -> p a b", a=16)
    bc16 = lambda ap: ap.unsqueeze(1).broadcast_to([128, 16, nh])
    ztot = sp.tile([128, 16 * nh], F32, tag="ztot")
    nc.vector.tensor_mul(b16(ztot), b16(ZmZb), bc16(Za_t))
    rzt = sp.tile([128, 16 * nh], F32, tag="rzt")
    nc.vector.reciprocal(rzt, ztot)
    c2 = sp.tile([128, 16 * nh], F32, tag="c2")
    nc.vector.tensor_sub(c2, ZmZb, X0R)
    c3 = sp.tile([128, 16 * nh], F32, tag="c3")
    nc.vector.tensor_mul(b16(c3), b16(c2), bc16(A0s))
    c4 = sp.tile([128, 16 * nh], F32, tag="c4")
    nc.vector.tensor_mul(b16(c4), b16(X0R), bc16(Za_t))
    corr = sp.tile([128, 16 * nh], F32, tag="corr")
    nc.vector.tensor_add(corr, c3, c4)

    # ---- inverse DFT fused with transpose-back: out_rows = P.T @ CINVZ -----
    with tc.tile_pool(name="fin", bufs=8) as fp, \
         tc.tile_pool(name="pso", bufs=7, space="PSUM") as pso:
        for l in range(16):
            for h in range(nh):
                tp = l * nh + h
                colsl = slice(l * bc + h * 128, l * bc + (h + 1) * 128)
                ps = pso.tile([128, 256], F32, tag="tro")
                nc.tensor.matmul(ps, Pc[:, colsl], CIZ, start=True, stop=False)
                nc.tensor.matmul(ps, Ps[:, colsl], SIZ, start=False, stop=True)
                nc.vector.tensor_copy(out=ps[:, 0:1], in_=corr[:, tp:tp + 1])
                fin = fp.tile([128, 256], F32, tag="fin")
                nc.scalar.activation(out=fin, in_=ps, func=AF.Ln,
                                     scale=rzt[:, tp:tp + 1])
                nc.sync.dma_start(
                    out=out_ap[h * 128:(h + 1) * 128, l, :],
                    in_=fin)

    _close_all()


def build_program(bc=BC):
    nc = bacc.Bacc("TRN2", target_bir_lowering=False, debug=False)
    logits = nc.dram_tensor("logits", [bc, 18, 256], F32, kind="ExternalInput").ap()
    out = nc.dram_tensor("out", [bc, 16, 256], F32, kind="ExternalOutput").ap()
    cnp = _consts()
    cdram = {k: nc.inline_tensor(v, name=f"c_{k.lower()}") for k, v in cnp.items()}
    with tile.TileContext(nc) as tc:
        _emit(tc, out, logits, cdram, bc)
    nc.compile()
    return nc


_CACHED = {}


def _get_program(bc=BC):
    if bc not in _CACHED:
        _CACHED[bc] = build_program(bc)
    return _CACHED[bc]


def run(logits, trace=False):
    logits = np.ascontiguousarray(logits, dtype=np.float32)
    assert logits.shape == (B_TOTAL, 18, 256), logits.shape
    nc = _get_program()
    in_maps = [{"logits": logits[i * BC:(i + 1) * BC]} for i in range(N_CORES)]
    res = run_bass_kernel_spmd(nc, in_maps, core_ids=list(range(N_CORES)), trace=trace)
    out = np.concatenate([r["out"] for r in res.results], axis=0)
    return out, res


def kernel(logits):
    out, _ = run(logits, trace=False)
    return out


# revision 17
# speedup vs baseline: 1.0576x; 1.0576x over previous
"""ASCADv2 head kernel for Trainium2 (8 NeuronCores, pure data parallel).

Algorithm (per batch element b; reference computes):
  probs = softmax(logits, -1); alpha=probs[0], beta=probs[1], ms=probs[2:]
  xorred[l,z] = sum_x ms[l,x] * beta[x^z]            (XOR convolution)
  out[l,z]    = sum_{x*y=z in GF(256)} inv_alpha[x] * xorred[l,y]
  return log(clip(out, 1e-12))

Key transforms used here:
  * XOR convolution diagonalizes under the Walsh-Hadamard transform H
    (constant +-1 256x256 matrix): xorred = H(( H m ) .* ( H beta ))/256.
  * The GF(256)* multiplicative convolution is a length-255 cyclic
    convolution in the discrete-log domain (generator g=3), diagonalized
    by a DFT-255 implemented as constant cos/sin matmuls; real-input
    conjugate symmetry halves the spectrum to k=0..127.
  * Softmax normalizers are factored out of the bilinear pipeline and
    re-applied as a per-row scale inside the final log (ACT: Ln(U*scale)).
  * z=0 column (the GF multiply-by-zero mass) is patched separately:
    out[l,0] = inv_a[0]*(sum_{y!=0} xorred[l,y]) + (sum inv_a)*xorred[l,0]
    with xorred[l,0] = <ms[l], beta> computed as a fused DVE dot product.

Layouts: softmax+log in rows-on-partition layout; all matmuls contract
over z so the bulk pipeline runs z-on-partition; PE transposes convert.
"""

import numpy as np

import concourse.bass as bass
import concourse.bacc as bacc
import concourse.tile as tile
import concourse.mybir as mybir
from concourse.bass_utils import run_bass_kernel_spmd

F32 = mybir.dt.float32
F32R = mybir.dt.float32r
AF = mybir.ActivationFunctionType
ALU = mybir.AluOpType

N_CORES = 8
B_TOTAL = 2048
BC = B_TOTAL // N_CORES  # batches per core

LOG_CLIP = float(np.log(np.float32(1e-12)))


# ----------------------------------------------------------------------------
# host-side constant tables
# ----------------------------------------------------------------------------

def _gf_tables():
    AES_POLY = 0x1B
    a = np.arange(256, dtype=np.int64)
    x = np.repeat(a, 256)
    y = np.tile(a, 256)
    r = np.zeros(256 * 256, dtype=np.int64)
    for _ in range(8):
        r ^= np.where((y & 1) != 0, x, 0)
        hi = (x & 0x80) != 0
        x = ((x << 1) & 0xFF) ^ np.where(hi, AES_POLY, 0)
        y >>= 1
    mult = r.reshape(256, 256)
    inv = np.argmax(mult == 1, axis=1)
    inv[0] = 0
    return mult, inv


def _consts():
    mult, inv = _gf_tables()
    # powers of generator 3 of GF(256)*
    powers = np.zeros(255, dtype=np.int64)
    v = 1
    for m in range(255):
        powers[m] = v
        v = mult[v, 3]
    assert v == 1

    H = np.array([[1.0]], dtype=np.float32)
    for _ in range(8):
        H = np.block([[H, H], [H, -H]]).astype(np.float32)

    # inverse WHT with dlog ordering fused: col m<255 -> xorred[g^m], col 255 -> xorred[0]
    HINVP = np.empty((256, 256), dtype=np.float32)
    HINVP[:, :255] = H[:, powers] / 256.0
    HINVP[:, 255] = H[:, 0] / 256.0

    # alpha permutation: Ag[k] = alpha[inv(g^k)]
    PINVP = np.zeros((256, 256), dtype=np.float32)
    for k in range(255):
        PINVP[inv[powers[k]], k] = 1.0
    PINVP[inv[0], 255] = 1.0  # unused row-255 output

    # forward DFT-255, half spectrum: cols 0..127 = cos, 128..255 = sin
    kf = np.arange(128)[None, :]
    j = np.arange(255)[:, None]
    CS = np.empty((255, 256), dtype=np.float32)
    CS[:, :128] = np.cos(2 * np.pi * j * kf / 255)
    CS[:, 128:] = np.sin(2 * np.pi * j * kf / 255)

    # inverse DFT with z-ordering fused; factor 2 for folded conjugate half
    m2 = np.arange(255)[None, :]
    kk = np.arange(128)[:, None]
    w = np.full((128, 1), 2.0, dtype=np.float32)
    w[0] = 1.0
    Ci = (w * np.cos(2 * np.pi * kk * m2 / 255) / 255).astype(np.float32)
    Si = (w * np.sin(2 * np.pi * kk * m2 / 255) / 255).astype(np.float32)
    CINVZ = np.zeros((128, 256), dtype=np.float32)
    SINVZ = np.zeros((128, 256), dtype=np.float32)
    CINVZ[:, powers] = Ci
    SINVZ[:, powers] = Si

    IDT = np.eye(128, dtype=np.float32)
    return dict(H=H, HINVP=HINVP, PINVP=PINVP, CS=CS, CINVZ=CINVZ,
                SINVZ=SINVZ, IDT=IDT)


# ----------------------------------------------------------------------------
# kernel emission
# ----------------------------------------------------------------------------

def _emit(tc, out_ap, logits_ap, cdram, bc, stage=None):
    nc = tc.nc
    nh = bc // 128            # 128-row chunks per l
    nt = 18 * nh              # (l,h) tiles
    nn = (16 * bc) // 512     # 512-wide column chunks over msbox
    lper = 512 // bc
    v3 = lambda ap: ap.rearrange("p (a b) -> p a b", a=lper)
    bcast = lambda ap: ap.unsqueeze(1).broadcast_to([128, lper, bc])
    b4 = lambda ap: ap.rearrange("p (a b) -> p a b", a=4 // nh) if nh < 4 else ap
    AX = mybir.AxisListType.X

    _cms = []
    def _reg(cm):
        _cms.append(cm)
        return cm
    def _close_all():
        for cm in reversed(_cms):
            try:
                cm.__exit__(None, None, None)
            except Exception:
                pass

    cpool_cm = _reg(tc.tile_pool(name="consts", bufs=1)); cpool = cpool_cm.__enter__()
    sp_cm = _reg(tc.tile_pool(name="small", bufs=1)); sp = sp_cm.__enter__()
    bigp_cm = _reg(tc.tile_pool(name="big", bufs=1)); bigp = bigp_cm.__enter__()
    xp_cm = _reg(tc.tile_pool(name="xin", bufs=2)); xp = xp_cm.__enter__()
    t4p_cm = _reg(tc.tile_pool(name="tmp4", bufs=3)); t4p = t4p_cm.__enter__()
    fp_cm = _reg(tc.tile_pool(name="fin", bufs=6)); fp = fp_cm.__enter__()
    # one PSUM pool, per-tag slots, total <= 8 banks
    psp_cm = _reg(tc.tile_pool(name="ps", bufs=1, space="PSUM")); psp = psp_cm.__enter__()

    def cload(name, rows, cols, src, dt=F32R):
        t = cpool.tile([rows, cols], dt, tag=name, name=name)
        nc.sync.dma_start(out=t, in_=src.bitcast(dt) if dt == F32R else src)
        return t

    HGd, HIPd, PIPd, CSd = cdram["H"].ap(), cdram["HINVP"].ap(), cdram["PINVP"].ap(), cdram["CS"].ap()
    HG = [cload(f"hg{k}", 128, 256, HGd[k * 128:(k + 1) * 128, :]) for k in range(2)]
    HIP = [cload(f"hip{k}", 128, 256, HIPd[k * 128:(k + 1) * 128, :]) for k in range(2)]
    PIP = [cload(f"pip{k}", 128, 256, PIPd[k * 128:(k + 1) * 128, :]) for k in range(2)]
    CSk = [cload("cs0", 128, 256, CSd[0:128, :]), cload("cs1", 127, 256, CSd[128:255, :])]
    CIZ = cload("ciz", 128, 256, cdram["CINVZ"].ap())
    SIZ = cload("siz", 128, 256, cdram["SINVZ"].ap())
    IDT = cload("idt", 128, 128, cdram["IDT"].ap(), dt=F32)

    Za_t = sp.tile([128, nh], F32, tag="Za_t")
    ZmZb = sp.tile([128, 16 * nh], F32, tag="ZmZb")
    X0R = sp.tile([128, 16 * nh], F32, tag="X0R")
    A0s = sp.tile([128, nh], F32, tag="A0s")
    rzt = sp.tile([128, 16 * nh], F32, tag="rzt")
    corr = sp.tile([128, 16 * nh], F32, tag="corr")

    ETmix = bigp.tile([128, nt, 2, 128], F32R, tag="ETmix")
    ETk = [ETmix[:, :, k, :] for k in range(2)]
    Wb = [sp.tile([128, bc], F32, tag=f"Wb{m}", name=f"Wb{m}") for m in range(2)]
    Ag = [sp.tile([128, bc], F32R, tag=f"Ag{m}", name=f"Ag{m}") for m in range(2)]
    Gc = sp.tile([128, bc], F32, tag="Gc")
    Gs = sp.tile([128, bc], F32, tag="Gs")
    V = [bigp.tile([128, 512], F32R, tag=f"V{m}", name=f"V{m}", bufs=3) for m in range(2)]
    # V is per-chunk now (consumed immediately by invWHT) -> small rotating tiles
    Xg = [bigp.tile([128, 16 * bc], F32R, tag=f"Xg{m}", name=f"Xg{m}") for m in range(2)]
    Pc = bigp.tile([128, 16 * bc], F32R, tag="Pc")
    Ps = bigp.tile([128, 16 * bc], F32R, tag="Ps")

    # ---- load rows, then per-(l,h)-pair transpose+exp ------------------------
    xrows = []
    for h in range(nh):
        X = xp.tile([128, 18, 256], F32, tag="X")
        nc.sync.dma_start(out=X, in_=logits_ap[h * 128:(h + 1) * 128, :, :])
        xrows.append(X)

    def trexp(tlist):
        # tlist: consecutive t indices (pairs) to transpose+exp
        for i in range(0, len(tlist), 2):
            ts2 = tlist[i:i + 2]
            ps = psp.tile([128, 512], F32, tag="tr", bufs=2, name="pstr")
            for j, t in enumerate(ts2):
                l, h = t // nh, t % nh
                for zc in range(2):
                    nc.tensor.transpose(
                        ps[:, j * 256 + zc * 128: j * 256 + (zc + 1) * 128],
                        xrows[h][:, l, zc * 128:(zc + 1) * 128], IDT)
            nc.scalar.activation(out=ETmix[:, ts2[0]:ts2[0] + len(ts2), :, :],
                                 in_=ps[:, 0:256 * len(ts2)], func=AF.Exp)

    def et_cols(k, t0, t1):
        return ETk[k][:, t0:t1, :]

    # alpha/beta first
    trexp(list(range(2 * nh)))

    # ---- alpha sum, beta WHT, alpha perm, G transform ------------------------
    psA = psp.tile([1, bc], F32, tag="mmC", name="psA", bufs=1)
    for k in range(2):
        nc.tensor.matmul(psA, HG[k][:, 0:1], et_cols(k, 0, nh),
                         start=(k == 0), stop=(k == 1))
    zarow = sp.tile([1, bc], F32, tag="zarow")
    nc.scalar.copy(out=zarow, in_=psA)
    for t in range(nh):
        nc.sync.dma_start(out=Za_t[:, t:t + 1],
                          in_=zarow[0:1, t * 128:(t + 1) * 128])

    for m in range(2):
        msl = slice(m * 128, (m + 1) * 128)
        ps = psp.tile([128, bc], F32, tag="mmw", bufs=1, name="psb")
        for k in range(2):
            nc.tensor.matmul(ps, HG[k][:, msl], et_cols(k, nh, 2 * nh),
                             start=(k == 0), stop=(k == 1))
        nc.scalar.copy(out=Wb[m], in_=ps)

    for m in range(2):
        msl = slice(m * 128, (m + 1) * 128)
        ps = psp.tile([128, bc], F32, tag="mmw", bufs=1, name="psb")
        for k in range(2):
            nc.tensor.matmul(ps, PIP[k][:, msl], et_cols(k, 0, nh),
                             start=(k == 0), stop=(k == 1))
        nc.scalar.copy(out=Ag[m], in_=ps)

    for dst, csl in ((Gc, slice(0, 128)), (Gs, slice(128, 256))):
        ps = psp.tile([128, bc], F32, tag="mmi", bufs=1, name="psg")
        nc.tensor.matmul(ps, CSk[0][:, csl], Ag[0], start=True, stop=False)
        nc.tensor.matmul(ps, CSk[1][:, csl], Ag[1][0:127, :], start=False, stop=True)
        nc.scalar.copy(out=dst, in_=ps)

    for t in range(nh):
        nc.sync.dma_start(out=A0s[:, t:t + 1],
                          in_=Ag[1][127:128, t * 128:(t + 1) * 128].bitcast(F32))

    # ---- chunk-interleaved main pipeline ------------------------------------
    tpn = 512 // 128
    for n in range(nn):
        nsl = slice(n * 512, (n + 1) * 512)
        t0 = 2 * nh + n * tpn
        # transpose+exp the 4 tiles of this chunk
        trexp(list(range(t0, t0 + tpn)))
        # WHT + Wb product -> V chunk
        vcur = []
        for m in range(2):
            msl = slice(m * 128, (m + 1) * 128)
            ps = psp.tile([128, 512], F32, tag="mmw", bufs=1, name="psw")
            for k in range(2):
                nc.tensor.matmul(ps, HG[k][:, msl], et_cols(k, t0, t0 + tpn),
                                 start=(k == 0), stop=(k == 1))
            vt = bigp.tile([128, 512], F32R, tag=f"V{m}", name=f"V{m}", bufs=3)
            nc.vector.tensor_mul(v3(vt), v3(ps), bcast(Wb[m]))
            vcur.append(vt)
            if m == 0:
                for q in range(tpn):
                    tq = n * tpn + q
                    nc.sync.dma_start(
                        out=ZmZb[:, tq:tq + 1],
                        in_=vt[0:1, q * 128:(q + 1) * 128].bitcast(F32))
        # inverse WHT -> Xg chunk
        for m in range(2):
            msl = slice(m * 128, (m + 1) * 128)
            ps = psp.tile([128, 512], F32, tag="mmi", bufs=1, name="psi")
            for k in range(2):
                nc.tensor.matmul(ps, HIP[k][:, msl], vcur[k],
                                 start=(k == 0), stop=(k == 1))
            if m == 0:
                nc.scalar.copy(out=Xg[m][:, nsl], in_=ps)
            else:
                nc.vector.tensor_copy(out=Xg[m][:, nsl], in_=ps)
                for q in range(tpn):
                    tq = n * tpn + q
                    nc.sync.dma_start(
                        out=X0R[:, tq:tq + 1],
                        in_=Xg[1][127:128, (n * tpn + q) * 128:(n * tpn + q + 1) * 128].bitcast(F32))
        # forward DFT + complex pointwise -> Pc/Ps chunk
        psC = psp.tile([128, 512], F32, tag="mmC", bufs=1, name="psC")
        nc.tensor.matmul(psC, CSk[0][:, 0:128], Xg[0][:, nsl], start=True, stop=False)
        nc.tensor.matmul(psC, CSk[1][:, 0:128], Xg[1][0:127, nsl], start=False, stop=True)
        psS = psp.tile([128, 512], F32, tag="mmS", bufs=1, name="psS")
        nc.tensor.matmul(psS, CSk[0][:, 128:256], Xg[0][:, nsl], start=True, stop=False)
        nc.tensor.matmul(psS, CSk[1][:, 128:256], Xg[1][0:127, nsl], start=False, stop=True)
        t1 = t4p.tile([128, 512], F32, tag="t1")
        t2 = t4p.tile([128, 512], F32, tag="t2")
        t3 = t4p.tile([128, 512], F32, tag="t3")
        t4 = t4p.tile([128, 512], F32, tag="t4")
        nc.vector.tensor_mul(v3(t1), v3(psC), bcast(Gc))
        nc.vector.tensor_mul(v3(t2), v3(psS), bcast(Gs))
        nc.vector.tensor_mul(v3(t3), v3(psC), bcast(Gs))
        nc.vector.tensor_mul(v3(t4), v3(psS), bcast(Gc))
        nc.gpsimd.tensor_sub(Pc[:, nsl], t1, t2)
        nc.gpsimd.tensor_add(Ps[:, nsl], t3, t4)
        # per-chunk corrections (needs ZmZb/X0R of this chunk + Za/A0)
        csl4 = slice(n * tpn, (n + 1) * tpn)
        nch = tpn // nh   # l-groups in chunk
        bx = lambda ap: ap.rearrange("p (a b) -> p a b", a=nch)
        bcx = lambda ap: ap.unsqueeze(1).broadcast_to([128, nch, nh])
        zt = sp.tile([128, tpn], F32, tag="zt")
        nc.vector.tensor_mul(bx(zt), bx(ZmZb[:, csl4]), bcx(Za_t))
        nc.vector.reciprocal(rzt[:, csl4], zt)
        cc2 = sp.tile([128, tpn], F32, tag="cc2")
        nc.vector.tensor_sub(cc2, ZmZb[:, csl4], X0R[:, csl4])
        cc3 = sp.tile([128, tpn], F32, tag="cc3")
        nc.vector.tensor_mul(bx(cc3), bx(cc2), bcx(A0s))
        cc4 = sp.tile([128, tpn], F32, tag="cc4")
        nc.vector.tensor_mul(bx(cc4), bx(X0R[:, csl4]), bcx(Za_t))
        nc.vector.tensor_add(corr[:, csl4], cc3, cc4)
        # inverse DFT fused with transpose-back + log, per tile of chunk
        for q in range(tpn):
            tq = n * tpn + q
            l, h = tq // nh, tq % nh
            colsl = slice((n * tpn + q) * 128, (n * tpn + q + 1) * 128)
            ps = psp.tile([128, 256], F32, tag="tro", bufs=2, name="pso")
            nc.tensor.matmul(ps, Pc[:, colsl], CIZ, start=True, stop=False)
            nc.tensor.matmul(ps, Ps[:, colsl], SIZ, start=False, stop=True)
            nc.vector.tensor_copy(out=ps[:, 0:1], in_=corr[:, tq:tq + 1])
            fin = fp.tile([128, 256], F32, tag="fin")
            nc.scalar.activation(out=fin, in_=ps, func=AF.Ln,
                                 scale=rzt[:, tq:tq + 1])
            nc.sync.dma_start(out=out_ap[h * 128:(h + 1) * 128, l, :], in_=fin)

    _close_all()


def build_program(bc=BC):
    nc = bacc.Bacc("TRN2", target_bir_lowering=False, debug=False)
    logits = nc.dram_tensor("logits", [bc, 18, 256], F32, kind="ExternalInput").ap()
    out = nc.dram_tensor("out", [bc, 16, 256], F32, kind="ExternalOutput").ap()
    cnp = _consts()
    cdram = {k: nc.inline_tensor(v, name=f"c_{k.lower()}") for k, v in cnp.items()}
    with tile.TileContext(nc) as tc:
        _emit(tc, out, logits, cdram, bc)
    nc.compile()
    return nc


_CACHED = {}


def _get_program(bc=BC):
    if bc not in _CACHED:
        _CACHED[bc] = build_program(bc)
    return _CACHED[bc]


def run(logits, trace=False):
    logits = np.ascontiguousarray(logits, dtype=np.float32)
    assert logits.shape == (B_TOTAL, 18, 256), logits.shape
    nc = _get_program()
    in_maps = [{"logits": logits[i * BC:(i + 1) * BC]} for i in range(N_CORES)]
    res = run_bass_kernel_spmd(nc, in_maps, core_ids=list(range(N_CORES)), trace=trace)
    out = np.concatenate([r["out"] for r in res.results], axis=0)
    return out, res


def kernel(logits):
    out, _ = run(logits, trace=False)
    return out
